# revision 13
# baseline (speedup 1.0000x reference)
"""Trainium2 Bass kernel for nn_CSPVLayer (GNN message passing), 8 NeuronCores.

Strategy: partition NODES across cores (6272/core, padded N=50176). Host sorts
edges by src node and assigns each edge to the core owning its src; scatter-mean
is then fully core-local. Per 128-node block, edges are padded to 128-multiples,
split into lo/hi dst-index windows (dma_gather idx is int16). Each core receives
only its [6272,256] f16 node-table shard as input; the full [50176,256] gather
table is built on-device with an AllGather (minimizes host->device bytes, which
dominate the measured time). Edge MLP runs feature-on-partition (W-stationary);
h[dst]/v[dst] rows arrive pre-transposed via dma_gather(transpose=True) from the
gathered table; h[src]/v[src] contributions come from a per-block indicator
matmul. Scatter is an indicator.T @ ef matmul accumulating sums+counts in PSUM.
Output is the f16 MLP delta only; the f32 residual add happens on host.
"""
import math
import numpy as np

N, E0, G, H, D = 50000, 400000, 256, 128, 128
NCORES = 8
NPAD = 50176            # 392 blocks of 128
NPC = NPAD // NCORES    # 6272 nodes/core
NBLK = NPC // 128       # 49 blocks/core
TBL = 32768             # gather-window rows (int16 idx limit)
WIN = NPAD - TBL        # hi window base = 17408
SPLIT = 25088           # dst < SPLIT -> lo window
CHUNK = 512


def _round128(x):
    return ((x + 127) // 128) * 128


def _pack_idx(vals, T):
    """int16 idx values [T] -> [32, T//16] wrapped layout, replicated x2
    (dma_gather queue 0 reads partitions 0-31 only)."""
    arr = np.zeros((32, T // 16), np.int16)
    cols = np.arange(T) // 16
    rows = np.arange(T) % 16
    for rep in range(2):
        arr[rows + 16 * rep, cols] = vals
    return arr


def kernel(**inputs):
    import concourse.bass as bass
    import concourse.bacc as bacc
    import concourse.mybir as mybir
    import concourse.tile as tile
    from concourse.bass_utils import run_bass_kernel_spmd

    f16, f32, i16 = mybir.dt.float16, mybir.dt.float32, mybir.dt.int16
    AF = mybir.ActivationFunctionType
    OP = mybir.AluOpType

    pos_diff = np.asarray(inputs["pos_diff"], np.float32)
    v = np.asarray(inputs["v"], np.float32)
    h = np.asarray(inputs["node_features"], np.float32)
    l = np.asarray(inputs["l"], np.float32)
    eni = np.asarray(inputs["edge_node_index"]).astype(np.int64)
    egi = np.asarray(inputs["edge_graph_index"]).astype(np.int64)
    E = pos_diff.shape[0]
    src, dst = eni[0], eni[1]

    # ---- weight algebra (host, exact f32 then cast) ----
    e_w1 = np.asarray(inputs["e_w1"], np.float32)
    W_hi = e_w1[0:128]
    W_hj = e_w1[128:256]
    W_l = e_w1[256:262]           # [6, H]
    W_v = e_w1[262:390]           # [128, H]
    W_pd = e_w1[390:518]
    vproj_w = np.asarray(inputs["vproj_w"], np.float32)
    vproj_b = np.asarray(inputs["vproj_b"], np.float32)
    Wv3 = vproj_w @ W_v           # [3, H]
    b1 = np.asarray(inputs["e_b1"], np.float32) + vproj_b @ W_v  # [H]
    Wlb = np.concatenate([W_l, b1[None, :]], 0)  # [7, H] (bias via l-row ones)
    e_w2 = np.asarray(inputs["e_w2"], np.float32)
    b2 = np.asarray(inputs["e_b2"], np.float32)
    n_w1 = np.asarray(inputs["n_w1"], np.float32)
    b3 = np.asarray(inputs["n_b1"], np.float32)
    n_w2 = np.asarray(inputs["n_w2"], np.float32)
    b4 = np.asarray(inputs["n_b2"], np.float32)

    # ---- node table shards (AllGathered on device) ----
    hv = np.zeros((NPAD, 256), np.float16)
    hv[:N, 0:128] = h.astype(np.float16)
    hv[:N, 128:131] = v.astype(np.float16)
    hv_own = np.stack([hv[c * NPC:(c + 1) * NPC] for c in range(NCORES)])
    ltab = np.zeros((G, 128), np.float16)
    ltab[:, 0:6] = l.astype(np.float16)
    ltab[:, 6] = 1.0              # ones row -> folds bias b1

    # ---- per-core edge partition, sort, block/seg grouping ----
    core_of = np.minimum(src // NPC, NCORES - 1)
    per_core = []  # list of dict(blk -> (lo_edges_idx, hi_edges_idx))
    for c in range(NCORES):
        sel = np.where(core_of == c)[0]
        s = sel[np.argsort(src[sel], kind="stable")]
        blk = (src[s] - c * NPC) // 128
        islo = dst[s] < SPLIT
        segs = {}
        for b in range(NBLK):
            m = blk == b
            segs[b] = (s[m & islo], s[m & ~islo])
        per_core.append(segs)

    caps = np.zeros((NBLK, 2), np.int64)
    for b in range(NBLK):
        for sgi in range(2):
            caps[b, sgi] = _round128(
                max(len(per_core[c][b][sgi]) for c in range(NCORES)))
    assert caps.sum(1).min() > 0, "empty block"
    # pad total to a 2048-multiple so the on-device sl transpose is legal
    caps[NBLK - 1, 1] += (-caps.sum()) % 2048
    T = int(caps.sum())
    T16 = T // 16

    # ---- packed per-core edge arrays ----
    idx_hv = np.zeros((NCORES, T), np.int64)
    idx_l = np.zeros((NCORES, T), np.int64)
    sl_row = np.full((NCORES, 1, T), -1.0, np.float16)
    posr = np.zeros((NCORES, 1, T), np.float32)
    for c in range(NCORES):
        off = 0
        for b in range(NBLK):
            for sgi in range(2):
                e_ids = per_core[c][b][sgi]
                n = len(e_ids)
                cp = int(caps[b, sgi])
                if n:
                    d = dst[e_ids]
                    idx_hv[c, off:off + n] = d if sgi == 0 else d - WIN
                    idx_l[c, off:off + n] = egi[e_ids]
                    sl_row[c, 0, off:off + n] = (src[e_ids] - c * NPC - b * 128
                                                 ).astype(np.float16)
                    posr[c, 0, off:off + n] = pos_diff[e_ids]
                off += cp
    idx2 = np.stack([
        np.concatenate([_pack_idx(idx_hv[c], T), _pack_idx(idx_l[c], T)], 1)
        for c in range(NCORES)])          # [NCORES, 32, 2*T16]
    sl2 = sl_row.reshape(NCORES, T // 128, 128)
    pos2 = posr.reshape(NCORES, 1, T)

    # ---- packed weights ----
    def blk128(rows, mat):
        out = np.zeros((128, 128), np.float32)
        out[rows[0]:rows[1]] = mat
        return out

    iota_row = np.tile(np.arange(128, dtype=np.float32)[None, :], (128, 1))
    small = np.zeros((128, 128), np.float32)
    small[:, 127] = 1.0           # ones column 127
    nwv3b = np.zeros((128, 128), np.float32)
    nwv3b[0:3] = -Wv3
    wv3b = np.zeros((128, 128), np.float32)
    wv3b[0:3] = Wv3
    wlbb = np.zeros((128, 128), np.float32)
    wlbb[0:7] = Wlb
    wpack16 = np.concatenate(
        [W_hi, W_hj, W_pd, e_w2, n_w1[0:128], n_w1[128:256], n_w2,
         iota_row, small, nwv3b, wv3b, wlbb], 1).astype(np.float16)
    freqs = np.exp(-np.log(10000.0) * np.arange(64, dtype=np.float64) / 64)
    freq2 = np.concatenate([freqs, freqs]).astype(np.float32)[None, :]  # [1,128]
    pht = np.concatenate([np.zeros(64), np.full(64, 0.25)]).astype(np.float32)
    wpack32 = np.stack(
        [b2, b3, b4, np.arange(128, dtype=np.float32), pht],
        1)                                             # [128, 5] f32

    # ================= build program =================
    nc = bacc.Bacc("TRN2", target_bir_lowering=False, debug=False,
                   num_devices=NCORES)

    def din(name, arr_one, dt=None):
        return nc.dram_tensor(name, list(arr_one.shape),
                              dt or mybir.dt.from_np(arr_one.dtype),
                              kind="ExternalInput").ap()

    import os as _os
    NOAG = bool(_os.environ.get("KNOAG"))
    KDBG = bool(_os.environ.get("KDBG"))
    DBGB = 1
    t_own = din("hv_own", hv_own[0])
    t_full_in = din("hv_fin", hv) if NOAG else None
    t_l = din("ltab", ltab)
    t_ix = din("idx2", idx2[0])
    t_sl = din("sl2", sl2[0])
    t_pos = din("pos2", pos2[0], mybir.dt.float32r)
    t_w16 = din("wpack16", wpack16)
    t_w32 = din("wpack32", wpack32)
    t_fq = din("freq2", freq2, mybir.dt.float32r)
    t_out = nc.dram_tensor("out", [NPC, 128], f16, kind="ExternalOutput").ap()
    t_dbg = {}
    if KDBG:
        capBd = int(caps[DBGB].sum())
        for nm, shp, dt_ in [
                ("d_tb", [128, 128], f16), ("d_gh", [128, 128], f16),
                ("d_gv", [128, 128], f16),
                ("d_slb", [1, capBd], f16), ("d_indT", [128, 512], f16),
                ("d_ef1", [128, 512], f16), ("d_gd", [128, 2, 512], f16),
                ("d_gl", [128, 1, 512], f16), ("d_pdT", [128, 512], f16),
                ("d_sums", [128, 129], f32), ("d_agg", [128, 128], f16),
                ("d_ind0", [128, 128], f16)]:
            t_dbg[nm] = nc.dram_tensor(nm, shp, dt_,
                                       kind="ExternalOutput").ap()

    with tile.TileContext(nc) as tc:
        import contextlib
        with contextlib.ExitStack() as ctx:
            cpool = ctx.enter_context(tc.tile_pool(name="consts", bufs=1))
            dpool = ctx.enter_context(
                tc.tile_pool(name="dram", bufs=1, space="DRAM"))
            bpool = ctx.enter_context(tc.tile_pool(name="blk", bufs=2))
            kpool = ctx.enter_context(tc.tile_pool(name="chk", bufs=3))
            tpool = ctx.enter_context(tc.tile_pool(name="tl", bufs=4))
            p1 = ctx.enter_context(tc.tile_pool(name="p1", bufs=2, space="PSUM"))
            pk = ctx.enter_context(tc.tile_pool(name="pk", bufs=2, space="PSUM"))
            ps = ctx.enter_context(tc.tile_pool(name="ps", bufs=2, space="PSUM"))
            pt = ctx.enter_context(tc.tile_pool(name="pt", bufs=2, space="PSUM"))

            # on-device AllGather of the node table
            if NOAG:
                t_lo = t_full_in[0:TBL]
                t_hi = t_full_in[WIN:WIN + TBL]
            else:
                hv_sh = dpool.tile([NPC, 256], f16)
                nc.gpsimd.dma_start(hv_sh[:], t_own[:])
                hv_full = dpool.tile([NPAD, 256], f16)
                nc.gpsimd.collective_compute(
                    "AllGather", mybir.AluOpType.bypass,
                    replica_groups=[list(range(NCORES))],
                    ins=[hv_sh[:].opt()], outs=[hv_full[:].opt()])
                t_lo = hv_full[0:TBL]
                t_hi = hv_full[WIN:WIN + TBL]

            W16 = cpool.tile([128, wpack16.shape[1]], f16, tag="w16")
            nc.sync.dma_start(out=W16[:], in_=t_w16[:])
            W = {k: W16[:, i * 128:(i + 1) * 128]
                 for i, k in enumerate(["W_hi", "W_hj", "W_pd", "e_w2",
                                        "n_w1a", "n_w1b", "n_w2",
                                        "iota_row", "small", "nwv3b",
                                        "wv3b", "wlbb"])}
            W32 = cpool.tile([128, wpack32.shape[1]], f32, tag="w32")
            nc.sync.dma_start(out=W32[:], in_=t_w32[:])
            b2c, b3c, b4c = (W32[:, 0:1], W32[:, 1:2], W32[:, 2:3])
            iota_col, phtc = W32[:, 3:4], W32[:, 4:5]
            fq = cpool.tile([1, 128], mybir.dt.float32r, tag="fq")
            nc.sync.dma_start(out=fq[:], in_=t_fq[:])
            ixt = cpool.tile([32, 2 * T16], i16, tag="ixt")
            nc.sync.dma_start(out=ixt[:], in_=t_ix[:])
            # xbar transposes corrupt each other when concurrent on this
            # stack -> strict barrier between each
            slcT16 = cpool.tile([128, T // 128], f16, tag="slcT16")
            nc.sync.dma_start_transpose(slcT16[:], t_sl[:])
            tc.strict_bb_all_engine_barrier()
            slcT = cpool.tile([128, T // 128], f32, tag="slcT")
            nc.vector.tensor_copy(slcT[:], slcT16[:])
            ones1 = cpool.tile([1, 128], f16, tag="ones1")
            nc.vector.tensor_scalar(out=ones1[:], in0=W16[0:1, 0:128],
                                    scalar1=0.0, scalar2=1.0,
                                    op0=OP.mult, op1=OP.add)
            ident = cpool.tile([128, 128], f16, tag="ident")
            nc.vector.tensor_scalar(out=ident[:], in0=W["iota_row"][:],
                                    scalar1=iota_col, scalar2=None,
                                    op0=OP.is_equal)

            # whole-shard transposed views of own h and v (xbar transpose
            # must not overlap the gather sprays -> do once, before barrier)
            hT = cpool.tile([128, NPC], f16, tag="hT")
            nc.sync.dma_start_transpose(hT[:], t_own[:, 0:128])
            tc.strict_bb_all_engine_barrier()
            vT = cpool.tile([128, NPC], f16, tag="vT")
            nc.sync.dma_start_transpose(vT[:], t_own[:, 128:256])

            # dma_gather's DRAM source AP is physical (untracked by Tile):
            # force ordering after the AllGather via a tracked probe read of
            # hv_full plus a strict barrier. The barrier also keeps the xbar
            # transposes above strictly before any dma_gather spray.
            if not NOAG:
                probe = cpool.tile([1, 128], f16, tag="probe")
                nc.gpsimd.dma_start(probe[:], hv_full[0:1, 0:128])
            tc.strict_bb_all_engine_barrier()

            for b in range(NBLK):
                capL, capH = int(caps[b, 0]), int(caps[b, 1])
                capB = capL + capH
                boff0 = int(caps[:b].sum())
                # --- own-node rows from the pre-transposed shard views ---
                g_h = hT[:, b * 128:(b + 1) * 128]
                g_v = vT[:, b * 128:(b + 1) * 128]
                ptb = pt.tile([128, 128], f32, tag="ptmp")
                nc.tensor.matmul(out=ptb[:], lhsT=g_h, rhs=W["W_hi"][:],
                                 start=True, stop=False)
                nc.tensor.matmul(out=ptb[:], lhsT=g_v[0:3, :],
                                 rhs=W["nwv3b"][0:3, :], start=False, stop=True)
                t_b = bpool.tile([128, 128], f16, tag="t_b")
                nc.scalar.activation(t_b[:], ptb[:], AF.Copy)
                if KDBG and b == DBGB:
                    nc.sync.dma_start(out=t_dbg["d_tb"][:], in_=t_b[:])
                    nc.sync.dma_start(out=t_dbg["d_gh"][:], in_=g_h)
                    nc.sync.dma_start(out=t_dbg["d_gv"][:], in_=g_v)

                # --- block loads ---
                sl_b = bpool.tile([1, capB], f16, tag="sl_b")
                nc.sync.dma_start(
                    out=sl_b[:],
                    in_=t_sl[boff0 // 128:(boff0 + capB) // 128, :])
                pos_b = bpool.tile([1, capB], mybir.dt.float32r, tag="pos_b")
                nc.sync.dma_start(
                    out=pos_b[:], in_=t_pos[0:1, boff0:boff0 + capB])
                if KDBG and b == DBGB:
                    nc.sync.dma_start(out=t_dbg["d_slb"][:], in_=sl_b[:])

                sums = ps.tile([128, 129], f32, tag="sums")
                first_sc = True
                boff = 0
                ntiles_blk = capB // 128
                tb_i = 0
                for sgi, cap in ((0, capL), (1, capH)):
                    tbl = t_lo if sgi == 0 else t_hi
                    done = 0
                    while done < cap:
                        Cc = min(CHUNK, cap - done)
                        o = boff0 + boff + done
                        g_dst = kpool.tile([128, 2, Cc], f16, tag="g_dst")
                        nc.gpsimd.dma_gather(
                            g_dst[:], tbl[:],
                            ixt[:, o // 16:(o + Cc) // 16], Cc, Cc, 256,
                            transpose=True)
                        g_l = kpool.tile([128, 1, Cc], f16, tag="g_l")
                        nc.gpsimd.dma_gather(
                            g_l[:], t_l[:],
                            ixt[:, T16 + o // 16:T16 + (o + Cc) // 16],
                            Cc, Cc, 128, transpose=True)
                        ob = boff + done   # offset within block arrays
                        pang = pk.tile([128, CHUNK], f32, tag="ktmp")
                        nc.tensor.matmul(
                            out=pang[:, :Cc], lhsT=fq[:],
                            rhs=pos_b[0:1, ob:ob + Cc],
                            start=True, stop=True)
                        q_t = kpool.tile([128, CHUNK], f32, tag="q_t")
                        nc.vector.tensor_scalar(
                            out=q_t[:, :Cc], in0=pang[:, :Cc],
                            scalar1=1.0 / (2.0 * math.pi), scalar2=phtc,
                            op0=OP.mult, op1=OP.add)
                        qi_t = kpool.tile([128, CHUNK], mybir.dt.int32,
                                          tag="qi_t")
                        nc.vector.tensor_copy(qi_t[:, :Cc], q_t[:, :Cc])
                        qf_t = kpool.tile([128, CHUNK], f32, tag="qf_t")
                        nc.vector.tensor_copy(qf_t[:, :Cc], qi_t[:, :Cc])
                        d_t = kpool.tile([128, CHUNK], f32, tag="d_t")
                        nc.vector.tensor_tensor(out=d_t[:, :Cc],
                                                in0=q_t[:, :Cc],
                                                in1=qf_t[:, :Cc],
                                                op=OP.subtract)
                        pdT = kpool.tile([128, CHUNK], f16, tag="pdT")
                        nc.scalar.activation(pdT[:, :Cc], d_t[:, :Cc], AF.Sin,
                                             scale=2.0 * math.pi)
                        psl = pk.tile([128, CHUNK], f32, tag="ktmp")
                        nc.tensor.matmul(out=psl[:, :Cc],
                                         lhsT=ones1[:],
                                         rhs=sl_b[0:1, ob:ob + Cc],
                                         start=True, stop=True)
                        indT = kpool.tile([128, CHUNK], f16, tag="indT")
                        nc.vector.tensor_scalar(out=indT[:, :Cc],
                                                in0=psl[:, :Cc],
                                                scalar1=iota_col,
                                                scalar2=None, op0=OP.is_equal)
                        ps1 = p1.tile([128, CHUNK], f32, tag="ps1")
                        nc.tensor.matmul(out=ps1[:, :Cc], lhsT=W["W_hj"][:],
                                         rhs=g_dst[:, 0, :Cc], start=True,
                                         stop=False)
                        nc.tensor.matmul(out=ps1[:, :Cc],
                                         lhsT=W["wv3b"][0:3, :],
                                         rhs=g_dst[0:3, 1, :Cc], start=False,
                                         stop=False)
                        nc.tensor.matmul(out=ps1[:, :Cc],
                                         lhsT=W["wlbb"][0:7, :],
                                         rhs=g_l[0:7, 0, :Cc], start=False,
                                         stop=False)
                        nc.tensor.matmul(out=ps1[:, :Cc], lhsT=W["W_pd"][:],
                                         rhs=pdT[:, :Cc], start=False,
                                         stop=False)
                        nc.tensor.matmul(out=ps1[:, :Cc], lhsT=t_b[:],
                                         rhs=indT[:, :Cc], start=False,
                                         stop=True)
                        ef1 = kpool.tile([128, CHUNK], f16, tag="ef1")
                        nc.scalar.activation(ef1[:, :Cc], ps1[:, :Cc], AF.Silu)
                        if KDBG and b == DBGB and boff == 0 and done == 0:
                            nc.sync.dma_start(out=t_dbg["d_indT"][:, :Cc],
                                              in_=indT[:, :Cc])
                            nc.sync.dma_start(out=t_dbg["d_ef1"][:, :Cc],
                                              in_=ef1[:, :Cc])
                            nc.sync.dma_start(out=t_dbg["d_gd"][:, :, :Cc],
                                              in_=g_dst[:, :, :Cc])
                            nc.sync.dma_start(out=t_dbg["d_gl"][:, :, :Cc],
                                              in_=g_l[:, :, :Cc])
                            nc.sync.dma_start(out=t_dbg["d_pdT"][:, :Cc],
                                              in_=pdT[:, :Cc])
                        for t in range(Cc // 128):
                            pe2 = pt.tile([128, 128], f32, tag="ptmp")
                            nc.tensor.matmul(out=pe2[:],
                                             lhsT=ef1[:, t * 128:(t + 1) * 128],
                                             rhs=W["e_w2"][:], start=True,
                                             stop=True)
                            ef2 = tpool.tile([128, 129], f16, tag="ef2")
                            nc.scalar.activation(ef2[:, 0:128], pe2[:], AF.Silu,
                                                 bias=b2c)
                            nc.vector.tensor_copy(ef2[:, 128:129],
                                                  W["small"][:, 127:128])
                            ind = tpool.tile([128, 128], f16, tag="ind")
                            nc.vector.tensor_scalar(
                                out=ind[:], in0=W["iota_row"][:],
                                scalar1=slcT[:, boff0 // 128 + tb_i:
                                             boff0 // 128 + tb_i + 1],
                                scalar2=None, op0=OP.is_equal)
                            nc.tensor.matmul(out=sums[:], lhsT=ind[:],
                                             rhs=ef2[:], start=first_sc,
                                             stop=(tb_i == ntiles_blk - 1))
                            if KDBG and b == DBGB and tb_i == 0:
                                nc.sync.dma_start(out=t_dbg["d_ind0"][:],
                                                  in_=ind[:])
                            first_sc = False
                            tb_i += 1
                        done += Cc
                    boff += cap

                # --- node MLP for this block ---
                inv = bpool.tile([128, 1], f32, tag="inv")
                nc.vector.tensor_scalar(out=inv[:], in0=sums[:, 128:129],
                                        scalar1=1.0, scalar2=None, op0=OP.max)
                inv2 = bpool.tile([128, 1], f32, tag="inv2")
                nc.vector.reciprocal(inv2[:], inv[:])
                agg = bpool.tile([128, 128], f16, tag="agg")
                nc.vector.tensor_scalar(out=agg[:], in0=sums[:, 0:128],
                                        scalar1=inv2[:], scalar2=None,
                                        op0=OP.mult)
                if KDBG and b == DBGB:
                    sc = bpool.tile([128, 129], f32, tag="sumsc")
                    nc.vector.tensor_copy(sc[:], sums[:])
                    nc.sync.dma_start(out=t_dbg["d_sums"][:], in_=sc[:])
                    nc.sync.dma_start(out=t_dbg["d_agg"][:], in_=agg[:])
                pat = pt.tile([128, 128], f16, tag="ptmp")
                nc.tensor.transpose(out=pat[:], in_=agg[:], identity=ident[:])
                aggT = bpool.tile([128, 128], f16, tag="aggT")
                nc.scalar.activation(aggT[:], pat[:], AF.Copy)
                p3 = pt.tile([128, 128], f32, tag="ptmp")
                nc.tensor.matmul(out=p3[:], lhsT=W["n_w1a"][:], rhs=g_h,
                                 start=True, stop=False)
                nc.tensor.matmul(out=p3[:], lhsT=W["n_w1b"][:], rhs=aggT[:],
                                 start=False, stop=True)
                o1 = bpool.tile([128, 128], f16, tag="o1")
                nc.scalar.activation(o1[:], p3[:], AF.Silu, bias=b3c)
                p4 = pt.tile([128, 128], f32, tag="ptmp")
                nc.tensor.matmul(out=p4[:], lhsT=W["n_w2"][:], rhs=o1[:],
                                 start=True, stop=True)
                o2 = bpool.tile([128, 128], f16, tag="o2")
                nc.scalar.activation(o2[:], p4[:], AF.Silu, bias=b4c)
                po = pt.tile([128, 128], f16, tag="ptmp")
                nc.tensor.transpose(out=po[:], in_=o2[:], identity=ident[:])
                ob2 = bpool.tile([128, 128], f16, tag="ob")
                nc.scalar.activation(ob2[:], po[:], AF.Copy)
                nc.sync.dma_start(out=t_out[b * 128:(b + 1) * 128, :],
                                  in_=ob2[:])

    nc.compile()

    in_maps = []
    for c in range(NCORES):
        m = dict(hv_own=hv_own[c], ltab=ltab, idx2=idx2[c], sl2=sl2[c],
                 pos2=pos2[c], wpack16=wpack16, wpack32=wpack32, freq2=freq2)
        if NOAG:
            m["hv_fin"] = hv
        in_maps.append(m)
    kr = run_bass_kernel_spmd(nc, in_maps, list(range(NCORES)))
    global LAST_RESULTS, LAST_NC, LAST_INMAPS
    LAST_RESULTS = kr
    LAST_NC = nc
    LAST_INMAPS = in_maps
    res = kr.results
    delta = np.concatenate([res[c]["out"] for c in range(NCORES)], 0)[:N]
    return h + delta.astype(np.float32)
